# revision 1
# baseline (speedup 1.0000x reference)
"""Self-attention kernel for TRN2, data-parallel over batch (8 cores).

Per core (one batch element):
  q/k/v projections from xT (built via cast + TensorE transpose),
  scores computed TRANSPOSED (sT[s,t] blocks) so softmax exp feeds the
  PV matmul without transposing the 2048x2048 attention matrix,
  row sums via a ones-column appended to v (free), normalization folded
  into the output-projection epilogue (per-partition scalar), residual
  added in fp32.

Matmul inputs bf16, PSUM accumulation fp32, softmax/normalize/residual fp32.
"""

import numpy as np

import concourse.bass as bass
import concourse.mybir as mybir
import concourse.tile as tile
from concourse import bacc
from concourse.bass import ds, ts
from concourse.bass_utils import run_bass_kernel_spmd
from concourse.masks import make_identity

F32 = mybir.dt.float32
BF16 = mybir.dt.bfloat16
AF = mybir.ActivationFunctionType

B, T, C, U, P = 8, 2048, 512, 256, 128
TC = T // P   # 16 row tiles
CCH = C // P  # 4 c-chunks
UCH = U // P  # 2 u-chunks
TBLK = 512    # t-block for attention
NTB = T // TBLK
SCALE = 1.0 / float(np.sqrt(U))

_cache = {}


def _build_kernel(tc):
    nc = tc.nc
    x = nc.dram_tensor("x", [T, C], F32, kind="ExternalInput").ap()
    Wq = nc.dram_tensor("Wq", [C, U], F32, kind="ExternalInput").ap()
    bq = nc.dram_tensor("bq", [U], F32, kind="ExternalInput").ap()
    Wk = nc.dram_tensor("Wk", [C, U], F32, kind="ExternalInput").ap()
    bk = nc.dram_tensor("bk", [U], F32, kind="ExternalInput").ap()
    Wv = nc.dram_tensor("Wv", [C, U], F32, kind="ExternalInput").ap()
    bv = nc.dram_tensor("bv", [U], F32, kind="ExternalInput").ap()
    Wa = nc.dram_tensor("Wa", [U, C], F32, kind="ExternalInput").ap()
    ba = nc.dram_tensor("ba", [C], F32, kind="ExternalInput").ap()
    out = nc.dram_tensor("out", [T, C], F32, kind="ExternalOutput").ap()

    consts = tc.alloc_tile_pool(name="consts", bufs=1)
    persist = tc.alloc_tile_pool(name="persist", bufs=1)

    # --- constants / weights (bf16 via casting SWDGE DMA) ---
    ones_row = consts.tile([1, P], BF16)
    nc.vector.memset(ones_row, 1.0)
    identity = consts.tile([P, P], BF16)
    make_identity(nc, identity)
    Wq_bf = consts.tile([P, CCH, U], BF16)
    nc.gpsimd.dma_start(out=Wq_bf, in_=Wq.rearrange("(cc p) u -> p cc u", p=P))
    Wk_bf = consts.tile([P, CCH, U], BF16)
    nc.gpsimd.dma_start(out=Wk_bf, in_=Wk.rearrange("(cc p) u -> p cc u", p=P))
    Wv_bf = consts.tile([P, CCH, U], BF16)
    nc.gpsimd.dma_start(out=Wv_bf, in_=Wv.rearrange("(cc p) u -> p cc u", p=P))
    Wa_bf = consts.tile([P, UCH, C], BF16)
    nc.gpsimd.dma_start(out=Wa_bf, in_=Wa.rearrange("(uc p) c -> p uc c", p=P))
    bq_sb = consts.tile([P, UCH], F32)
    nc.sync.dma_start(out=bq_sb, in_=bq.rearrange("(uc p) -> p uc", p=P))
    bk_sb = consts.tile([P, UCH], F32)
    nc.sync.dma_start(out=bk_sb, in_=bk.rearrange("(uc p) -> p uc", p=P))
    bv_bf = consts.tile([1, U], BF16)
    nc.gpsimd.dma_start(out=bv_bf, in_=bv[None, :])
    ba_bf = consts.tile([1, C], BF16)
    nc.gpsimd.dma_start(out=ba_bf, in_=ba[None, :])

    # persistent layout tensors
    x_sb = persist.tile([P, TC, C], F32)      # x rows (residual + transpose src)
    xT_sb = persist.tile([P, CCH, T], BF16)   # x^T  (c on partitions)
    qT_sb = persist.tile([P, UCH, T], BF16)   # q^T  (u on partitions)
    kT_sb = persist.tile([P, UCH, T], BF16)   # k^T
    v_sb = persist.tile([P, TC, U + 1], BF16)  # v row-major + ones column
    aT_sb = persist.tile([P, UCH, T], BF16)   # a^T (unnormalized)
    nc.vector.memset(v_sb[:, :, U:U + 1], 1.0)

    with tc.tile_pool(name="warm", bufs=1, space="PSUM") as warm_pool:
        wtile = warm_pool.tile([P, P], F32, name="warmup")
        for i in range(36):
            nc.tensor.matmul(wtile, lhsT=identity, rhs=identity,
                             start=(i == 0), stop=(i == 35))

    for tt in range(TC):
        eng = nc.sync if tt % 2 == 0 else nc.scalar
        eng.dma_start(out=x_sb[:, tt, :], in_=x[ts(tt, P), :])

    # --- phase 1: xT via DVE cast + TensorE transpose ---
    with tc.tile_pool(name="xbf", bufs=4) as xbf_pool, \
         tc.tile_pool(name="tpsum", bufs=4, space="PSUM") as tpsum:
        for tt in range(TC):
            x_bf = xbf_pool.tile([P, C], BF16, tag="xbf")
            nc.vector.tensor_copy(out=x_bf, in_=x_sb[:, tt, :])
            for cc in range(CCH):
                tps = tpsum.tile([P, P], BF16, tag="tps")
                nc.tensor.transpose(tps, x_bf[:, ts(cc, P)], identity)
                nc.vector.tensor_copy(out=xT_sb[:, cc, ts(tt, P)], in_=tps)

    # --- phase 2: projections ---
    with tc.tile_pool(name="wpsum", bufs=2, space="PSUM") as wpsum, \
         tc.tile_pool(name="vpsum", bufs=2, space="PSUM") as vpsum:
        for (WT, bias_sb, dst) in ((Wq_bf, bq_sb, qT_sb), (Wk_bf, bk_sb, kT_sb)):
            for uc in range(UCH):
                for tb in range(NTB):
                    ps = wpsum.tile([P, TBLK], F32, tag="wps")
                    for cc in range(CCH):
                        nc.tensor.matmul(
                            ps,
                            lhsT=WT[:, cc, ts(uc, P)],
                            rhs=xT_sb[:, cc, ds(tb * TBLK, TBLK)],
                            start=(cc == 0),
                            stop=(cc == CCH - 1),
                        )
                    nc.scalar.activation(
                        out=dst[:, uc, ds(tb * TBLK, TBLK)],
                        in_=ps,
                        func=AF.Identity,
                        bias=bias_sb[:, uc:uc + 1],
                        scale=1.0,
                    )
        for tt in range(TC):
            ps = vpsum.tile([P, U], F32, tag="vps")
            for cc in range(CCH):
                nc.tensor.matmul(
                    ps,
                    lhsT=xT_sb[:, cc, ts(tt, P)],
                    rhs=Wv_bf[:, cc, :],
                    start=(cc == 0),
                    stop=False,
                )
            nc.tensor.matmul(ps, lhsT=ones_row, rhs=bv_bf, start=False, stop=True)
            nc.vector.tensor_copy(out=v_sb[:, tt, 0:U], in_=ps)

    # --- phase 3: attention per t-block ---
    spsum = tc.alloc_tile_pool(name="spsum", bufs=2, space="PSUM")
    apsum = tc.alloc_tile_pool(name="apsum", bufs=4, space="PSUM")
    ypsum = tc.alloc_tile_pool(name="ypsum", bufs=2, space="PSUM")
    p_pool = tc.alloc_tile_pool(name="p_pool", bufs=TC + 1)
    a_pool = tc.alloc_tile_pool(name="a_pool", bufs=4)
    rcp_pool = tc.alloc_tile_pool(name="rcp_pool", bufs=9)
    y_pool = tc.alloc_tile_pool(name="y_pool", bufs=3)

    deferred = [None]

    def finish(tb, rcps):
        for tsl in range(NTB):
            row0 = tb * TBLK + tsl * P
            yps = ypsum.tile([P, C], F32, tag="yps")
            for uc in range(UCH):
                nc.tensor.matmul(
                    yps,
                    lhsT=aT_sb[:, uc, ds(row0, P)],
                    rhs=Wa_bf[:, uc, :],
                    start=(uc == 0),
                    stop=False,
                )
            nc.tensor.matmul(yps, lhsT=ones_row, rhs=ba_bf, start=False, stop=True)
            y_sb = y_pool.tile([P, C], F32, tag="ysb")
            nc.vector.tensor_scalar(
                out=y_sb, in0=yps, scalar1=rcps[tsl], scalar2=None,
                op0=mybir.AluOpType.mult,
            )
            nc.vector.tensor_add(out=y_sb, in0=y_sb, in1=x_sb[:, tb * NTB + tsl, :])
            nc.sync.dma_start(out=out[ds(row0, P), :], in_=y_sb)

    def pv_col(sc, apss):
        for tsl in range(NTB):
            nc.tensor.matmul(
                apss[tsl],
                lhsT=pts[sc][:, ts(tsl, P)],
                rhs=v_sb[:, sc, :],
                start=(sc == 0),
                stop=(sc == TC - 1),
            )

    for tb in range(NTB):
        # scores (transposed) + exp, with PV trailing 2 stages behind
        pts = []
        apss = [apsum.tile([P, U + 1], F32, tag="aps", name=f"aps{tb}_{i}") for i in range(NTB)]
        for sc in range(TC):
            sps = spsum.tile([P, TBLK], F32, tag="sps")
            for uc in range(UCH):
                nc.tensor.matmul(
                    sps,
                    lhsT=kT_sb[:, uc, ts(sc, P)],
                    rhs=qT_sb[:, uc, ds(tb * TBLK, TBLK)],
                    start=(uc == 0),
                    stop=(uc == UCH - 1),
                )
            pt = p_pool.tile([P, TBLK], BF16, tag="pt")
            nc.scalar.activation(out=pt, in_=sps, func=AF.Exp, scale=SCALE)
            pts.append(pt)
            if sc >= 2:
                pv_col(sc - 2, apss)
        pv_col(TC - 2, apss)
        pv_col(TC - 1, apss)
        # drain psum: recip of row sums + bf16 cast + TensorE transpose to aT
        rcps = []
        for tsl in range(NTB):
            aps = apss[tsl]
            rcp = rcp_pool.tile([P, 1], F32, tag="rcp")
            nc.vector.reciprocal(rcp, aps[:, U:U + 1])
            rcps.append(rcp)
            a_bf = a_pool.tile([P, U], BF16, tag="abf")
            nc.vector.tensor_copy(out=a_bf, in_=aps[:, 0:U])
            for uc in range(UCH):
                tps = spsum.tile([P, P], BF16, tag="sps")
                nc.tensor.transpose(tps, a_bf[:, ts(uc, P)], identity)
                nc.vector.tensor_copy(
                    out=aT_sb[:, uc, ds(tb * TBLK + tsl * P, P)], in_=tps,
                )
        # deferred output projection of the previous block (hides aT latency)
        if deferred[0] is not None:
            finish(*deferred[0])
        deferred[0] = (tb, rcps)
    finish(*deferred[0])

    for pool in (y_pool, rcp_pool, a_pool, p_pool,
                 ypsum, apsum, spsum, persist, consts):
        pool.release()


def _get_nc():
    if "nc" not in _cache:
        nc = bacc.Bacc("TRN2", target_bir_lowering=False, debug=False)
        with tile.TileContext(nc) as tc:
            _build_kernel(tc)
        nc.compile()
        _cache["nc"] = nc
    return _cache["nc"]


def kernel(**inputs):
    nc = _get_nc()
    shared = {k: np.ascontiguousarray(np.asarray(v, dtype=np.float32))
              for k, v in inputs.items() if k != "x"}
    xs = np.ascontiguousarray(np.asarray(inputs["x"], dtype=np.float32))
    in_maps = [dict(shared, x=xs[b]) for b in range(B)]
    res = run_bass_kernel_spmd(nc, in_maps, core_ids=list(range(B)))
    return np.stack([res.results[b]["out"] for b in range(B)], axis=0)



# revision 4
# speedup vs baseline: 1.0168x; 1.0168x over previous
"""Self-attention kernel for TRN2, data-parallel over batch (8 cores).

Per core (one batch element), fp8e4 DoubleRow matmuls throughout:
  x loaded fp32 (residual) + cast bf16 -> TensorE transpose -> xT fp8.
  q/k/v projections fp8 DoubleRow (contraction 2x128 per pass), biases:
  q/k per-partition via DVE epilogue, v via ones-row matmul.
  Scores computed TRANSPOSED (sT[s,t]) with u-pairs in one DR matmul;
  exp on ScalarE over 2-bank psum groups (merged, scale=1/sqrt(U),
  shift -2 for fp8 range), p stored fp8.
  PV with v as stationary (s-pairs) emits aT[u,t] directly -- no
  attention-matrix transposes.  Row sums via all-ones DR matmul
  replicated across partitions; 1/D folded into the aT psum->sbuf copy,
  so the output projection epilogue is a single residual add in fp32.
"""

import numpy as np

import concourse.bass as bass
import concourse.mybir as mybir
import concourse.tile as tile
from concourse import bacc
from concourse.bass import ds, ts
from concourse.bass_utils import run_bass_kernel_spmd
from concourse.masks import make_identity

F32 = mybir.dt.float32
BF16 = mybir.dt.bfloat16
F8 = mybir.dt.float8e4
AF = mybir.ActivationFunctionType
DR = mybir.MatmulPerfMode.DoubleRow

B, T, C, U, P = 8, 2048, 512, 256, 128
TC = T // P   # 16 row tiles
CCH = C // P  # 4 c-chunks
UCH = U // P  # 2 u-chunks
TBLK = 512    # t-block for attention
NTB = T // TBLK
SCALE = 1.0 / float(np.sqrt(U))
SHIFT = -2.0  # exp(x*SCALE + SHIFT): keeps p in fp8e4 range

USE_DR = True

_cache = {}


def _dr_matmul(nc, out, lhsT3, rhs3, start, stop):
    """One fp8 DoubleRow matmul over [K,2,M]x[K,2,N], or two plain matmuls."""
    if USE_DR:
        nc.tensor.matmul(out, lhsT=lhsT3, rhs=rhs3, start=start, stop=stop,
                         perf_mode=DR)
    else:
        nc.tensor.matmul(out, lhsT=lhsT3[:, 0], rhs=rhs3[:, 0],
                         start=start, stop=False)
        nc.tensor.matmul(out, lhsT=lhsT3[:, 1], rhs=rhs3[:, 1],
                         start=False, stop=stop)


def _build_kernel(tc):
    nc = tc.nc
    x = nc.dram_tensor("x", [T, C], F32, kind="ExternalInput").ap()
    Wq = nc.dram_tensor("Wq", [C, U], F32, kind="ExternalInput").ap()
    bq = nc.dram_tensor("bq", [U], F32, kind="ExternalInput").ap()
    Wk = nc.dram_tensor("Wk", [C, U], F32, kind="ExternalInput").ap()
    bk = nc.dram_tensor("bk", [U], F32, kind="ExternalInput").ap()
    Wv = nc.dram_tensor("Wv", [C, U], F32, kind="ExternalInput").ap()
    bv = nc.dram_tensor("bv", [U], F32, kind="ExternalInput").ap()
    Wa = nc.dram_tensor("Wa", [U, C], F32, kind="ExternalInput").ap()
    ba = nc.dram_tensor("ba", [C], F32, kind="ExternalInput").ap()
    out = nc.dram_tensor("out", [T, C], F32, kind="ExternalOutput").ap()

    consts = tc.alloc_tile_pool(name="consts", bufs=1)
    persist = tc.alloc_tile_pool(name="persist", bufs=1)

    # --- constants ---
    identity = consts.tile([P, P], BF16)
    make_identity(nc, identity)
    ones_row = consts.tile([1, P], F8)
    nc.vector.memset(ones_row, 1.0)
    ones_pair = consts.tile([P, 2, P], F8)
    nc.vector.memset(ones_pair, 1.0)
    shift_col = consts.tile([P, 1], F32)
    nc.vector.memset(shift_col, SHIFT)

    # --- weights: casting DMA fp32->bf16, then DVE cast -> fp8 ---
    # layouts: W* [c_lo, cc, u] so cc-pairs (0,1),(2,3) give c/c+128 pairs;
    # Wa [u_lo, uc, c] matching aT's (u_lo, uc) partition layout.
    Wq_s = consts.tile([P, CCH, U], F8)
    Wk_s = consts.tile([P, CCH, U], F8)
    Wv_s = consts.tile([P, CCH, U], F8)
    Wa_s = consts.tile([P, UCH, C], F8)
    bq_sb = consts.tile([P, UCH], F32)
    nc.sync.dma_start(out=bq_sb, in_=bq.rearrange("(uc p) -> p uc", p=P))
    bk_sb = consts.tile([P, UCH], F32)
    nc.sync.dma_start(out=bk_sb, in_=bk.rearrange("(uc p) -> p uc", p=P))
    bv_row = consts.tile([1, U], F8)
    ba_row = consts.tile([1, C], F8)

    with tc.tile_pool(name="wstage", bufs=1) as wstage:
        Wq_bf = wstage.tile([P, CCH, U], BF16, tag="wq")
        nc.gpsimd.dma_start(out=Wq_bf, in_=Wq.rearrange("(cc p) u -> p cc u", p=P))
        nc.vector.tensor_copy(out=Wq_s, in_=Wq_bf)
        Wk_bf = wstage.tile([P, CCH, U], BF16, tag="wk")
        nc.gpsimd.dma_start(out=Wk_bf, in_=Wk.rearrange("(cc p) u -> p cc u", p=P))
        nc.vector.tensor_copy(out=Wk_s, in_=Wk_bf)
        Wv_bf = wstage.tile([P, CCH, U], BF16, tag="wv")
        nc.gpsimd.dma_start(out=Wv_bf, in_=Wv.rearrange("(cc p) u -> p cc u", p=P))
        nc.vector.tensor_copy(out=Wv_s, in_=Wv_bf)
        Wa_bf = wstage.tile([P, UCH, C], BF16, tag="wa")
        nc.gpsimd.dma_start(out=Wa_bf, in_=Wa.rearrange("(uc p) c -> p uc c", p=P))
        nc.vector.tensor_copy(out=Wa_s, in_=Wa_bf)
        bv_bf = wstage.tile([1, U], BF16, tag="bv")
        nc.gpsimd.dma_start(out=bv_bf, in_=bv[None, :])
        nc.vector.tensor_copy(out=bv_row, in_=bv_bf)
        ba_bf = wstage.tile([1, C], BF16, tag="ba")
        nc.gpsimd.dma_start(out=ba_bf, in_=ba[None, :])
        nc.vector.tensor_copy(out=ba_row, in_=ba_bf)

        # --- persistent tensors ---
        x_sb = persist.tile([P, TC, C], F32)      # residual + transpose source
        xT_sb = persist.tile([P, CCH, T], F8)     # x^T  (c on partitions)
        qT_sb = persist.tile([P, UCH, T], F8)     # q^T  (u on partitions)
        kT_sb = persist.tile([P, UCH, T], F8)
        v_sb = persist.tile([P, TC, U], F8)       # v row-major (s, u)
        aT_sb = persist.tile([P, UCH, T], F8)     # a^T normalized
        p_sb = [persist.tile([P, TC, TBLK], F8, name=f"p{i}") for i in range(NTB)]

        # x loads (fp32, HW DGE, alternate queues)
        for tt in range(TC):
            eng = nc.sync if tt % 2 == 0 else nc.scalar
            eng.dma_start(out=x_sb[:, tt, :], in_=x[ts(tt, P), :])

        # HAM warmup while DMAs land
        with tc.tile_pool(name="warm", bufs=1, space="PSUM") as warm_pool:
            wtile = warm_pool.tile([P, P], F32, name="warmup")
            for i in range(36):
                nc.tensor.matmul(wtile, lhsT=identity, rhs=identity,
                                 start=(i == 0), stop=(i == 35))

        # --- phase 1+2: transpose + projections, per t-block group ---
        with tc.tile_pool(name="xbf", bufs=4) as xbf_pool, \
             tc.tile_pool(name="tpsum", bufs=2, space="PSUM") as tpsum, \
             tc.tile_pool(name="wpsum", bufs=2, space="PSUM") as wpsum, \
             tc.tile_pool(name="vpsum", bufs=2, space="PSUM") as vpsum:
            for g in range(NTB):
                for tt in range(4 * g, 4 * g + 4):
                    x_bf = xbf_pool.tile([P, C], BF16, tag="xbf")
                    nc.vector.tensor_copy(out=x_bf, in_=x_sb[:, tt, :])
                    tps = tpsum.tile([P, CCH, P], BF16, tag="tps")
                    for cc in range(CCH):
                        nc.tensor.transpose(tps[:, cc, :], x_bf[:, ts(cc, P)],
                                            identity)
                    nc.vector.tensor_copy(out=xT_sb[:, :, ts(tt, P)], in_=tps)
                    # v projection for this row tile
                    vps = vpsum.tile([P, U], F32, tag="vps")
                    for cp in range(2):
                        _dr_matmul(nc, vps,
                                   xT_sb[:, ds(2 * cp, 2), ts(tt, P)],
                                   Wv_s[:, ds(2 * cp, 2), :],
                                   start=(cp == 0), stop=False)
                    nc.tensor.matmul(vps, lhsT=ones_row, rhs=bv_row,
                                     start=False, stop=True)
                    nc.vector.tensor_copy(out=v_sb[:, tt, :], in_=vps)
                # q/k projections for this 512-wide t block
                for (W_s, b_sb, dst) in ((Wk_s, bk_sb, kT_sb),
                                         (Wq_s, bq_sb, qT_sb)):
                    for uc in range(UCH):
                        wps = wpsum.tile([P, TBLK], F32, tag="wps")
                        for cp in range(2):
                            _dr_matmul(nc, wps,
                                       W_s[:, ds(2 * cp, 2), ts(uc, P)],
                                       xT_sb[:, ds(2 * cp, 2), ts(g, TBLK)],
                                       start=(cp == 0), stop=(cp == 1))
                        nc.vector.tensor_scalar_add(
                            out=dst[:, uc, ts(g, TBLK)], in0=wps,
                            scalar1=b_sb[:, uc:uc + 1])

        # --- phase 3: attention ---
        sps_pool = tc.alloc_tile_pool(name="sps", bufs=2, space="PSUM")
        pv_pool = tc.alloc_tile_pool(name="pvps", bufs=2, space="PSUM")
        d_pool = tc.alloc_tile_pool(name="dps", bufs=1, space="PSUM")
        y_psum = tc.alloc_tile_pool(name="ypsum", bufs=1, space="PSUM")
        rcp_pool = tc.alloc_tile_pool(name="rcp", bufs=2)
        y_pool = tc.alloc_tile_pool(name="y", bufs=3)

        def outproj(tb, tsl):
            row0 = tb * TBLK + tsl * P
            yps = y_psum.tile([P, C], F32, tag="yps")
            _dr_matmul(nc, yps, aT_sb[:, :, ds(row0, P)], Wa_s,
                       start=True, stop=False)
            nc.tensor.matmul(yps, lhsT=ones_row, rhs=ba_row,
                             start=False, stop=True)
            y_sb = y_pool.tile([P, C], F32, tag="ysb")
            nc.vector.tensor_add(out=y_sb, in0=yps,
                                 in1=x_sb[:, tb * NTB + tsl, :])
            nc.sync.dma_start(out=out[ds(row0, P), :], in_=y_sb)

        def pv_pair(tb, j, apsT, drep):
            rhs_p = p_sb[tb][:, ds(2 * j, 2), :]
            for uc in range(UCH):
                _dr_matmul(nc, apsT[uc],
                           v_sb[:, ds(2 * j, 2), ts(uc, P)], rhs_p,
                           start=(j == 0), stop=(j == 7))
            _dr_matmul(nc, drep, ones_pair, rhs_p,
                       start=(j == 0), stop=(j == 7))

        for tb in range(NTB):
            apsT = [pv_pool.tile([P, TBLK], F32, tag="pv",
                                 name=f"apsT{tb}_{uc}") for uc in range(UCH)]
            drep = d_pool.tile([P, TBLK], F32, tag="d", name=f"drep{tb}")
            for j in range(8):
                sps_t = sps_pool.tile([P, 2, TBLK], F32, tag="sps")
                for h in range(2):
                    _dr_matmul(nc, sps_t[:, h, :],
                               kT_sb[:, :, ts(2 * j + h, P)],
                               qT_sb[:, :, ts(tb, TBLK)],
                               start=True, stop=True)
                nc.scalar.activation(out=p_sb[tb][:, ds(2 * j, 2), :],
                                     in_=sps_t, func=AF.Exp,
                                     bias=shift_col, scale=SCALE)
                if j >= 2:
                    pv_pair(tb, j - 2, apsT, drep)
                if tb > 0 and 2 <= j < 6:
                    outproj(tb - 1, j - 2)
            pv_pair(tb, 6, apsT, drep)
            pv_pair(tb, 7, apsT, drep)
            # drain: reciprocal of replicated row sums, normalize into aT
            rcp = rcp_pool.tile([P, TBLK], F32, tag="rcp")
            nc.vector.reciprocal(rcp, drep)
            for uc in range(UCH):
                nc.vector.tensor_mul(out=aT_sb[:, uc, ts(tb, TBLK)],
                                     in0=apsT[uc], in1=rcp)
        for tsl in range(NTB):
            outproj(NTB - 1, tsl)

        for pool in (y_pool, rcp_pool, y_psum, d_pool, pv_pool, sps_pool):
            pool.release()
    for pool in (persist, consts):
        pool.release()


def _get_nc():
    if "nc" not in _cache:
        nc = bacc.Bacc("TRN2", target_bir_lowering=False, debug=False)
        with tile.TileContext(nc) as tc:
            _build_kernel(tc)
        nc.compile()
        _cache["nc"] = nc
    return _cache["nc"]


def kernel(**inputs):
    nc = _get_nc()
    shared = {k: np.ascontiguousarray(np.asarray(v, dtype=np.float32))
              for k, v in inputs.items() if k != "x"}
    xs = np.ascontiguousarray(np.asarray(inputs["x"], dtype=np.float32))
    in_maps = [dict(shared, x=xs[b]) for b in range(B)]
    res = run_bass_kernel_spmd(nc, in_maps, core_ids=list(range(B)))
    return np.stack([res.results[b]["out"] for b in range(B)], axis=0)


# revision 7
# speedup vs baseline: 1.3505x; 1.3282x over previous
"""Self-attention kernel for TRN2, data-parallel over batch (8 cores).

Per core (one batch element), fp8e4 DoubleRow matmuls throughout:
  x loaded fp32 (residual) -> gpsimd cast bf16 -> TensorE transpose ->
  DVE copy to xT fp8.
  q/k/v projections fp8 DoubleRow (contraction 2x128 per pass); q/k bias
  via ScalarE Identity epilogue (per-partition), v bias via ones-row
  matmul, v copy via ScalarE.
  Scores computed TRANSPOSED (sT[s,t]) with u-pairs in one DR matmul;
  exp on ScalarE over 2-bank psum groups (scale 1/sqrt(U), shift -2 for
  fp8 range), p stored fp8.
  PV with v as stationary (s-pairs) emits aT[u,t] directly -- no
  attention-matrix transposes.  Row sums via all-ones DR matmul
  replicated across partitions; 1/D (fast reciprocal) folded into the
  aT psum->sbuf copy.  Output projection is a single DR matmul; residual
  x+ba is pre-combined on gpsimd so the epilogue is one DVE add.
"""

import numpy as np

import concourse.bass as bass
import concourse.mybir as mybir
import concourse.tile as tile
from concourse import bacc
from concourse.bass import ds, ts
from concourse.bass_utils import run_bass_kernel_spmd
from concourse.masks import make_identity

F32 = mybir.dt.float32
BF16 = mybir.dt.bfloat16
F8 = mybir.dt.float8e4
AF = mybir.ActivationFunctionType
DR = mybir.MatmulPerfMode.DoubleRow

B, T, C, U, P = 8, 2048, 512, 256, 128
TC = T // P   # 16 row tiles
CCH = C // P  # 4 c-chunks
UCH = 2       # u-chunks
TBLK = 512    # t-block for attention
NTB = T // TBLK
SCALE = 1.0 / float(np.sqrt(U))
SHIFT = -2.0  # exp(x*SCALE + SHIFT): keeps p in fp8e4 range

USE_DR = True

_cache = {}


def _dr_matmul(nc, out, lhsT3, rhs3, start, stop):
    """One fp8 DoubleRow matmul over [K,2,M]x[K,2,N], or two plain matmuls."""
    if USE_DR:
        nc.tensor.matmul(out, lhsT=lhsT3, rhs=rhs3, start=start, stop=stop,
                         perf_mode=DR)
    else:
        nc.tensor.matmul(out, lhsT=lhsT3[:, 0], rhs=rhs3[:, 0],
                         start=start, stop=False)
        nc.tensor.matmul(out, lhsT=lhsT3[:, 1], rhs=rhs3[:, 1],
                         start=False, stop=stop)


def _build_kernel(tc):
    nc = tc.nc
    x = nc.dram_tensor("x", [T, C], F32, kind="ExternalInput").ap()
    Wq = nc.dram_tensor("Wq", [C, U], F32, kind="ExternalInput").ap()
    bq = nc.dram_tensor("bq", [U], F32, kind="ExternalInput").ap()
    Wk = nc.dram_tensor("Wk", [C, U], F32, kind="ExternalInput").ap()
    bk = nc.dram_tensor("bk", [U], F32, kind="ExternalInput").ap()
    Wv = nc.dram_tensor("Wv", [C, U], F32, kind="ExternalInput").ap()
    bv = nc.dram_tensor("bv", [U], F32, kind="ExternalInput").ap()
    Wa = nc.dram_tensor("Wa", [U, C], F32, kind="ExternalInput").ap()
    ba = nc.dram_tensor("ba", [C], F32, kind="ExternalInput").ap()
    out = nc.dram_tensor("out", [T, C], F32, kind="ExternalOutput").ap()

    consts = tc.alloc_tile_pool(name="consts", bufs=1)
    persist = tc.alloc_tile_pool(name="persist", bufs=1)

    # --- constants ---
    identity = consts.tile([P, P], BF16)
    make_identity(nc, identity)
    ones_row = consts.tile([1, P], F8)
    nc.vector.memset(ones_row, 1.0)
    ones_pair = consts.tile([P, 2, P], F8)
    nc.vector.memset(ones_pair, 1.0)
    shift_col = consts.tile([P, 1], F32)
    nc.vector.memset(shift_col, SHIFT)

    # --- persistent tensors ---
    x_sb = persist.tile([P, TC, C], F32)      # transpose source
    x_res = persist.tile([P, TC, C], F32)     # x + ba (residual)
    xT_sb = persist.tile([P, CCH, T], F8)     # x^T  (c on partitions)
    qT_sb = persist.tile([P, UCH, T], F8)     # q^T  (u on partitions)
    kT_sb = persist.tile([P, UCH, T], F8)
    v_sb = persist.tile([P, TC, U], F8)       # v row-major (s, u)
    aT_sb = persist.tile([P, UCH, T], F8)     # a^T normalized
    p_sb = [persist.tile([P, TC, TBLK], F8, name=f"p{i}") for i in range(NTB)]

    # x loads (fp32, HW DGE, alternate queues)
    for tt in range(TC):
        eng = nc.sync if tt % 2 == 0 else nc.scalar
        eng.dma_start(out=x_sb[:, tt, :], in_=x[ts(tt, P), :])

    # --- weights: fp32 DMA (HW DGE, vector queue) + DVE cast -> fp8 ---
    # layouts: W* [c_lo, cc, u] so cc-pairs (0,1),(2,3) give c/c+128 pairs;
    # Wa [u_lo, uc, c] matching aT's (u_lo, uc) partition layout.
    Wq_s = consts.tile([P, CCH, U], F8)
    Wk_s = consts.tile([P, CCH, U], F8)
    Wv_s = consts.tile([P, CCH, U], F8)
    Wa_s = consts.tile([P, UCH, C], F8)
    bq_sb = consts.tile([P, UCH], F32)
    nc.sync.dma_start(out=bq_sb, in_=bq.rearrange("(uc p) -> p uc", p=P))
    bk_sb = consts.tile([P, UCH], F32)
    nc.sync.dma_start(out=bk_sb, in_=bk.rearrange("(uc p) -> p uc", p=P))
    bv_row = consts.tile([1, U], F8)
    ba_col = consts.tile([P, C], F32)   # ba on partition 0
    ba_bc = consts.tile([P, C], F32)    # ba broadcast to all partitions

    with tc.tile_pool(name="wstage", bufs=1) as wstage:
        Wv_f = wstage.tile([P, CCH, U], F32, tag="wv")
        nc.sync.dma_start(out=Wv_f, in_=Wv.rearrange("(cc p) u -> p cc u", p=P))
        nc.vector.tensor_copy(out=Wv_s, in_=Wv_f)
        Wk_f = wstage.tile([P, CCH, U], F32, tag="wk")
        nc.sync.dma_start(out=Wk_f, in_=Wk.rearrange("(cc p) u -> p cc u", p=P))
        nc.vector.tensor_copy(out=Wk_s, in_=Wk_f)
        Wq_f = wstage.tile([P, CCH, U], F32, tag="wq")
        nc.sync.dma_start(out=Wq_f, in_=Wq.rearrange("(cc p) u -> p cc u", p=P))
        nc.vector.tensor_copy(out=Wq_s, in_=Wq_f)
        Wa_f = wstage.tile([P, UCH, C], F32, tag="wa")
        nc.sync.dma_start(out=Wa_f, in_=Wa.rearrange("(uc p) c -> p uc c", p=P))
        nc.vector.tensor_copy(out=Wa_s, in_=Wa_f)
        bv_f = wstage.tile([1, U], F32, tag="bv")
        nc.sync.dma_start(out=bv_f, in_=bv[None, :])
        nc.vector.tensor_copy(out=bv_row, in_=bv_f)
        nc.sync.dma_start(out=ba_col[0:1, :], in_=ba[None, :])

        # HAM warmup while DMAs land
        with tc.tile_pool(name="warm", bufs=1, space="PSUM") as warm_pool:
            wtile = warm_pool.tile([P, P], F32, name="warmup")
            for i in range(36):
                nc.tensor.matmul(wtile, lhsT=identity, rhs=identity,
                                 start=(i == 0), stop=(i == 35))

        # --- phase 1+2: transpose + projections, per t-block group ---
        with tc.tile_pool(name="xbf", bufs=4) as xbf_pool, \
             tc.tile_pool(name="tpsum", bufs=2, space="PSUM") as tpsum, \
             tc.tile_pool(name="wpsum", bufs=2, space="PSUM") as wpsum, \
             tc.tile_pool(name="vpsum", bufs=2, space="PSUM") as vpsum:
            for g in range(NTB):
                for tt in range(4 * g, 4 * g + 4):
                    x_bf = xbf_pool.tile([P, C], BF16, tag="xbf")
                    nc.gpsimd.tensor_copy(out=x_bf, in_=x_sb[:, tt, :])
                    tps = tpsum.tile([P, CCH, P], BF16, tag="tps")
                    for cc in range(CCH):
                        nc.tensor.transpose(tps[:, cc, :], x_bf[:, ts(cc, P)],
                                            identity)
                    nc.vector.tensor_copy(out=xT_sb[:, :, ts(tt, P)], in_=tps)
                    # v projection for this row tile
                    vps = vpsum.tile([P, U], F32, tag="vps")
                    for cp in range(2):
                        _dr_matmul(nc, vps,
                                   xT_sb[:, ds(2 * cp, 2), ts(tt, P)],
                                   Wv_s[:, ds(2 * cp, 2), :],
                                   start=(cp == 0), stop=False)
                    nc.tensor.matmul(vps, lhsT=ones_row, rhs=bv_row,
                                     start=False, stop=True)
                    nc.scalar.copy(out=v_sb[:, tt, :], in_=vps)
                # q/k projections for this 512-wide t block
                for (W_s, b_sb, dst) in ((Wk_s, bk_sb, kT_sb),
                                         (Wq_s, bq_sb, qT_sb)):
                    for uc in range(UCH):
                        wps = wpsum.tile([P, TBLK], F32, tag="wps")
                        for cp in range(2):
                            _dr_matmul(nc, wps,
                                       W_s[:, ds(2 * cp, 2), ts(uc, P)],
                                       xT_sb[:, ds(2 * cp, 2), ts(g, TBLK)],
                                       start=(cp == 0), stop=(cp == 1))
                        nc.scalar.activation(out=dst[:, uc, ts(g, TBLK)],
                                             in_=wps, func=AF.Identity,
                                             bias=b_sb[:, uc:uc + 1], scale=1.0)
            # residual x + ba on gpsimd (lags; consumed late)
            nc.gpsimd.partition_broadcast(out_ap=ba_bc, in_ap=ba_col,
                                          channels=P)
            for tt in range(TC):
                nc.gpsimd.tensor_add(out=x_res[:, tt, :],
                                     in0=x_sb[:, tt, :], in1=ba_bc)

    # --- phase 3: attention ---
    sps_pool = tc.alloc_tile_pool(name="sps", bufs=2, space="PSUM")
    pv_pool = tc.alloc_tile_pool(name="pvps", bufs=2, space="PSUM")
    d_pool = tc.alloc_tile_pool(name="dps", bufs=1, space="PSUM")
    y_psum = tc.alloc_tile_pool(name="ypsum", bufs=1, space="PSUM")
    rcp_pool = tc.alloc_tile_pool(name="rcp", bufs=2)
    y_pool = tc.alloc_tile_pool(name="y", bufs=3)

    def outproj(tb, tsl):
        row0 = tb * TBLK + tsl * P
        yps = y_psum.tile([P, C], F32, tag="yps")
        _dr_matmul(nc, yps, aT_sb[:, :, ds(row0, P)], Wa_s,
                   start=True, stop=True)
        y_sb = y_pool.tile([P, C], F32, tag="ysb")
        nc.vector.tensor_add(out=y_sb, in0=yps,
                             in1=x_res[:, tb * NTB + tsl, :])
        nc.sync.dma_start(out=out[ds(row0, P), :], in_=y_sb)

    def pv_pair(tb, j, apsT, drep):
        rhs_p = p_sb[tb][:, ds(2 * j, 2), :]
        for uc in range(UCH):
            _dr_matmul(nc, apsT[uc],
                       v_sb[:, ds(2 * j, 2), ts(uc, P)], rhs_p,
                       start=(j == 0), stop=(j == 7))
        _dr_matmul(nc, drep, ones_pair, rhs_p,
                   start=(j == 0), stop=(j == 7))

    for tb in range(NTB):
        apsT = [pv_pool.tile([P, TBLK], F32, tag="pv",
                             name=f"apsT{tb}_{uc}") for uc in range(UCH)]
        drep = d_pool.tile([P, TBLK], F32, tag="d", name=f"drep{tb}")
        for j in range(8):
            sps_t = sps_pool.tile([P, 2, TBLK], F32, tag="sps")
            for h in range(2):
                _dr_matmul(nc, sps_t[:, h, :],
                           kT_sb[:, :, ts(2 * j + h, P)],
                           qT_sb[:, :, ts(tb, TBLK)],
                           start=True, stop=True)
            nc.scalar.activation(out=p_sb[tb][:, ds(2 * j, 2), :],
                                 in_=sps_t, func=AF.Exp,
                                 bias=shift_col, scale=SCALE)
            if j >= 2:
                pv_pair(tb, j - 2, apsT, drep)
            if tb > 0 and 2 <= j < 6:
                outproj(tb - 1, j - 2)
        pv_pair(tb, 6, apsT, drep)
        pv_pair(tb, 7, apsT, drep)
        # drain: fast reciprocal of replicated row sums, normalize into aT
        rcp = rcp_pool.tile([P, TBLK], F32, tag="rcp")
        nc.vector.reciprocal_approx_fast(out=rcp, in_=drep)
        for uc in range(UCH):
            nc.vector.tensor_mul(out=aT_sb[:, uc, ts(tb, TBLK)],
                                 in0=apsT[uc], in1=rcp)
    for tsl in range(NTB):
        outproj(NTB - 1, tsl)

    for pool in (y_pool, rcp_pool, y_psum, d_pool, pv_pool, sps_pool,
                 persist, consts):
        pool.release()


def _get_nc():
    if "nc" not in _cache:
        nc = bacc.Bacc("TRN2", target_bir_lowering=False, debug=False)
        with tile.TileContext(nc) as tc:
            _build_kernel(tc)
        nc.compile()
        _cache["nc"] = nc
    return _cache["nc"]


def kernel(**inputs):
    nc = _get_nc()
    shared = {k: np.ascontiguousarray(np.asarray(v, dtype=np.float32))
              for k, v in inputs.items() if k != "x"}
    xs = np.ascontiguousarray(np.asarray(inputs["x"], dtype=np.float32))
    in_maps = [dict(shared, x=xs[b]) for b in range(B)]
    res = run_bass_kernel_spmd(nc, in_maps, core_ids=list(range(B)))
    return np.stack([res.results[b]["out"] for b in range(B)], axis=0)


# revision 14
# speedup vs baseline: 1.4364x; 1.0636x over previous
"""Self-attention kernel for TRN2, data-parallel over batch (8 cores).

Per core (one batch element), fp8e4 DoubleRow matmuls throughout:
  x loaded fp32 (residual) -> gpsimd cast bf16 -> TensorE transpose ->
  DVE copy to xT fp8.
  q/k/v projections fp8 DoubleRow (contraction 2x128 per pass); q/k bias
  via ScalarE Identity epilogue (per-partition), v bias via ones-row
  matmul, v copy via ScalarE.
  Scores computed TRANSPOSED (sT[s,t]) with u-pairs in one DR matmul;
  exp on ScalarE over 2-bank psum groups (scale 1/sqrt(U), shift -2 for
  fp8 range), p stored fp8.
  PV with v as stationary (s-pairs) emits aT[u,t] directly -- no
  attention-matrix transposes.  Row sums via all-ones DR matmul
  replicated across partitions; 1/D (fast reciprocal) folded into the
  aT psum->sbuf copy.  Output projection is a single DR matmul; residual
  x+ba is pre-combined on gpsimd so the epilogue is one DVE add.
"""

import numpy as np

import concourse.bass as bass
import concourse.mybir as mybir
import concourse.tile as tile
from concourse import bacc
from concourse.bass import ds, ts
from concourse.bass_utils import run_bass_kernel_spmd
from concourse.masks import make_identity

F32 = mybir.dt.float32
BF16 = mybir.dt.bfloat16
F8 = mybir.dt.float8e4
AF = mybir.ActivationFunctionType
DR = mybir.MatmulPerfMode.DoubleRow

B, T, C, U, P = 8, 2048, 512, 256, 128
TC = T // P   # 16 row tiles
CCH = C // P  # 4 c-chunks
UCH = 2       # u-chunks
TBLK = 512    # t-block for attention
NTB = T // TBLK
SCALE = 1.0 / float(np.sqrt(U))
SHIFT = -2.0  # exp(x*SCALE + SHIFT): keeps p in fp8e4 range

USE_DR = True

_cache = {}


def _dr_matmul(nc, out, lhsT3, rhs3, start, stop):
    """One fp8 DoubleRow matmul over [K,2,M]x[K,2,N], or two plain matmuls."""
    if USE_DR:
        nc.tensor.matmul(out, lhsT=lhsT3, rhs=rhs3, start=start, stop=stop,
                         perf_mode=DR)
    else:
        nc.tensor.matmul(out, lhsT=lhsT3[:, 0], rhs=rhs3[:, 0],
                         start=start, stop=False)
        nc.tensor.matmul(out, lhsT=lhsT3[:, 1], rhs=rhs3[:, 1],
                         start=False, stop=stop)


def _build_kernel(tc):
    nc = tc.nc
    x = nc.dram_tensor("x", [T, C], F32, kind="ExternalInput").ap()
    Wq = nc.dram_tensor("Wq", [C, U], F32, kind="ExternalInput").ap()
    bq = nc.dram_tensor("bq", [U], F32, kind="ExternalInput").ap()
    Wk = nc.dram_tensor("Wk", [C, U], F32, kind="ExternalInput").ap()
    bk = nc.dram_tensor("bk", [U], F32, kind="ExternalInput").ap()
    Wv = nc.dram_tensor("Wv", [C, U], F32, kind="ExternalInput").ap()
    bv = nc.dram_tensor("bv", [U], F32, kind="ExternalInput").ap()
    Wa = nc.dram_tensor("Wa", [U, C], F32, kind="ExternalInput").ap()
    ba = nc.dram_tensor("ba", [C], F32, kind="ExternalInput").ap()
    out = nc.dram_tensor("out", [T, C], F32, kind="ExternalOutput").ap()

    consts = tc.alloc_tile_pool(name="consts", bufs=1)
    persist = tc.alloc_tile_pool(name="persist", bufs=1)

    # --- constants ---
    identity = consts.tile([P, P], BF16)
    make_identity(nc, identity)
    ones_row = consts.tile([1, P], F8)
    nc.vector.memset(ones_row, 1.0)
    ones_pair = consts.tile([P, 2, P], F8)
    nc.vector.memset(ones_pair, 1.0)
    shift_col = consts.tile([P, 1], F32)
    nc.vector.memset(shift_col, SHIFT)

    # --- persistent tensors ---
    x_sb = persist.tile([P, TC, C], BF16)     # transpose source + residual
    xT_sb = persist.tile([P, CCH, T], F8)     # x^T  (c on partitions)
    qT_sb = persist.tile([P, UCH, T], F8)     # q^T  (u on partitions)
    kT_sb = persist.tile([P, UCH, T], F8)
    v_sb = persist.tile([P, TC, U], F8)       # v row-major (s, u)
    aT_sb = persist.tile([P, UCH, T], F8)     # a^T normalized
    p_sb = [persist.tile([P, TC, TBLK], F8, name=f"p{i}") for i in range(NTB)]

    # x loads: SWDGE casting DMA fp32->bf16, 2 row tiles per descriptor batch
    for i in range(8):
        nc.gpsimd.dma_start(
            out=x_sb[:, ds(2 * i, 2), :],
            in_=x.rearrange("(tc p) c -> p tc c", p=P)[:, ds(2 * i, 2), :])

    # --- weights: fp32 DMA (HW DGE, vector queue) + DVE cast -> fp8 ---
    # layouts: W* [c_lo, cc, u] so cc-pairs (0,1),(2,3) give c/c+128 pairs;
    # Wa [u_lo, uc, c] matching aT's (u_lo, uc) partition layout.
    Wq_s = consts.tile([P, CCH, U], F8)
    Wk_s = consts.tile([P, CCH, U], F8)
    Wv_s = consts.tile([P, CCH, U], F8)
    Wa_s = consts.tile([P, UCH, C], F8)
    bq_sb = consts.tile([P, UCH], F32)
    nc.sync.dma_start(out=bq_sb, in_=bq.rearrange("(uc p) -> p uc", p=P))
    bk_sb = consts.tile([P, UCH], F32)
    nc.sync.dma_start(out=bk_sb, in_=bk.rearrange("(uc p) -> p uc", p=P))
    bv_row = consts.tile([1, U], F8)
    ba_row = consts.tile([1, C], F8)

    with tc.tile_pool(name="wstage", bufs=1) as wstage:
        Wv_f = wstage.tile([P, CCH, U], F32, tag="wv")
        nc.sync.dma_start(out=Wv_f, in_=Wv.rearrange("(cc p) u -> p cc u", p=P))
        nc.vector.tensor_copy(out=Wv_s, in_=Wv_f)
        Wk_f = wstage.tile([P, CCH, U], F32, tag="wk")
        nc.sync.dma_start(out=Wk_f, in_=Wk.rearrange("(cc p) u -> p cc u", p=P))
        nc.vector.tensor_copy(out=Wk_s, in_=Wk_f)
        Wq_f = wstage.tile([P, CCH, U], F32, tag="wq")
        nc.sync.dma_start(out=Wq_f, in_=Wq.rearrange("(cc p) u -> p cc u", p=P))
        nc.vector.tensor_copy(out=Wq_s, in_=Wq_f)
        Wa_f = wstage.tile([P, UCH, C], F32, tag="wa")
        nc.sync.dma_start(out=Wa_f, in_=Wa.rearrange("(uc p) c -> p uc c", p=P))
        nc.vector.tensor_copy(out=Wa_s, in_=Wa_f)
        bv_f = wstage.tile([1, U], F32, tag="bv")
        nc.sync.dma_start(out=bv_f, in_=bv[None, :])
        nc.vector.tensor_copy(out=bv_row, in_=bv_f)
        ba_f = wstage.tile([1, C], F32, tag="ba")
        nc.sync.dma_start(out=ba_f, in_=ba[None, :])
        nc.vector.tensor_copy(out=ba_row, in_=ba_f)

        # HAM warmup while DMAs land
        with tc.tile_pool(name="warm", bufs=1, space="PSUM") as warm_pool:
            wtile = warm_pool.tile([P, P], F32, name="warmup")
            for i in range(36):
                nc.tensor.matmul(wtile, lhsT=identity, rhs=identity,
                                 start=(i == 0), stop=(i == 35))

        # --- phase 1+2: transpose + projections, per t-block group ---
        with tc.tile_pool(name="xbf", bufs=4) as xbf_pool, \
             tc.tile_pool(name="tpsum", bufs=2, space="PSUM") as tpsum, \
             tc.tile_pool(name="wpsum", bufs=2, space="PSUM") as wpsum, \
             tc.tile_pool(name="vpsum", bufs=2, space="PSUM") as vpsum:
            for g in range(NTB):
                for tt in range(4 * g, 4 * g + 4):
                    tps = tpsum.tile([P, CCH, P], BF16, tag="tps")
                    for cc in range(CCH):
                        nc.tensor.transpose(tps[:, cc, :],
                                            x_sb[:, tt, ts(cc, P)], identity)
                    nc.vector.tensor_copy(out=xT_sb[:, :, ts(tt, P)], in_=tps)
                    # v projection for this row tile
                    vps = vpsum.tile([P, U], F32, tag="vps")
                    for cp in range(2):
                        _dr_matmul(nc, vps,
                                   xT_sb[:, ds(2 * cp, 2), ts(tt, P)],
                                   Wv_s[:, ds(2 * cp, 2), :],
                                   start=(cp == 0), stop=False)
                    nc.tensor.matmul(vps, lhsT=ones_row, rhs=bv_row,
                                     start=False, stop=True)
                    nc.scalar.copy(out=v_sb[:, tt, :], in_=vps)
                # q/k projections for this 512-wide t block
                for (W_s, b_sb, dst) in ((Wk_s, bk_sb, kT_sb),
                                         (Wq_s, bq_sb, qT_sb)):
                    for uc in range(UCH):
                        wps = wpsum.tile([P, TBLK], F32, tag="wps")
                        for cp in range(2):
                            _dr_matmul(nc, wps,
                                       W_s[:, ds(2 * cp, 2), ts(uc, P)],
                                       xT_sb[:, ds(2 * cp, 2), ts(g, TBLK)],
                                       start=(cp == 0), stop=(cp == 1))
                        nc.scalar.activation(out=dst[:, uc, ts(g, TBLK)],
                                             in_=wps, func=AF.Identity,
                                             bias=b_sb[:, uc:uc + 1], scale=1.0)


    # --- phase 3: attention ---
    sps_pool = tc.alloc_tile_pool(name="sps", bufs=2, space="PSUM")
    pv_pool = tc.alloc_tile_pool(name="pvps", bufs=2, space="PSUM")
    d_pool = tc.alloc_tile_pool(name="dps", bufs=1, space="PSUM")
    y_psum = tc.alloc_tile_pool(name="ypsum", bufs=1, space="PSUM")
    rcp_pool = tc.alloc_tile_pool(name="rcp", bufs=2)
    y_pool = tc.alloc_tile_pool(name="y", bufs=3)

    def outproj(tb, tsl, alt_pool=None):
        row0 = tb * TBLK + tsl * P
        if alt_pool is not None:
            yps = alt_pool.tile([P, C], F32, tag="d", name=f"yalt{tsl}")
        else:
            yps = y_psum.tile([P, C], F32, tag="yps")
        _dr_matmul(nc, yps, aT_sb[:, :, ds(row0, P)], Wa_s,
                   start=True, stop=False)
        nc.tensor.matmul(yps, lhsT=ones_row, rhs=ba_row,
                         start=False, stop=True)
        y_sb = y_pool.tile([P, C], F32, tag="ysb")
        nc.vector.tensor_add(out=y_sb, in0=yps,
                             in1=x_sb[:, tb * NTB + tsl, :])
        nc.sync.dma_start(out=out[ds(row0, P), :], in_=y_sb)

    def pv_pair(tb, j, apsT, drep):
        rhs_p = p_sb[tb][:, ds(2 * j, 2), :]
        for uc in range(UCH):
            _dr_matmul(nc, apsT[uc],
                       v_sb[:, ds(2 * j, 2), ts(uc, P)], rhs_p,
                       start=(j == 0), stop=(j == 7))
        _dr_matmul(nc, drep, ones_pair, rhs_p,
                   start=(j == 0), stop=(j == 7))

    for tb in range(NTB):
        apsT = [pv_pool.tile([P, TBLK], F32, tag="pv",
                             name=f"apsT{tb}_{uc}") for uc in range(UCH)]
        drep = d_pool.tile([P, TBLK], F32, tag="d", name=f"drep{tb}")
        for j in range(8):
            sps_t = sps_pool.tile([P, 2, TBLK], F32, tag="sps")
            for h in range(2):
                _dr_matmul(nc, sps_t[:, h, :],
                           kT_sb[:, :, ts(2 * j + h, P)],
                           qT_sb[:, :, ts(tb, TBLK)],
                           start=True, stop=True)
            nc.scalar.activation(out=p_sb[tb][:, ds(2 * j, 2), :],
                                 in_=sps_t, func=AF.Exp,
                                 bias=shift_col, scale=SCALE)
            if j >= 2:
                pv_pair(tb, j - 2, apsT, drep)
            if tb > 0 and 2 <= j < 6:
                outproj(tb - 1, j - 2)
        pv_pair(tb, 6, apsT, drep)
        pv_pair(tb, 7, apsT, drep)
        # drain: fast reciprocal of replicated row sums, normalize into aT
        rcp = rcp_pool.tile([P, TBLK], F32, tag="rcp")
        nc.vector.reciprocal_approx_fast(out=rcp, in_=drep)
        for uc in range(UCH):
            nc.vector.tensor_mul(out=aT_sb[:, uc, ts(tb, TBLK)],
                                 in0=apsT[uc], in1=rcp)
    for tsl in range(NTB):
        # alternate with the freed D bank so the tail pipelines
        outproj(NTB - 1, tsl, alt_pool=d_pool if tsl % 2 == 1 else None)

    for pool in (y_pool, rcp_pool, y_psum, d_pool, pv_pool, sps_pool,
                 persist, consts):
        pool.release()


def _get_nc():
    if "nc" not in _cache:
        nc = bacc.Bacc("TRN2", target_bir_lowering=False, debug=False)
        with tile.TileContext(nc) as tc:
            _build_kernel(tc)
        nc.compile()
        _cache["nc"] = nc
    return _cache["nc"]


def kernel(**inputs):
    nc = _get_nc()
    shared = {k: np.ascontiguousarray(np.asarray(v, dtype=np.float32))
              for k, v in inputs.items() if k != "x"}
    xs = np.ascontiguousarray(np.asarray(inputs["x"], dtype=np.float32))
    in_maps = [dict(shared, x=xs[b]) for b in range(B)]
    res = run_bass_kernel_spmd(nc, in_maps, core_ids=list(range(B)))
    return np.stack([res.results[b]["out"] for b in range(B)], axis=0)


# revision 17
# speedup vs baseline: 1.4603x; 1.0166x over previous
"""Self-attention kernel for TRN2, data-parallel over batch (8 cores).

Per core (one batch element), fp8e4 DoubleRow matmuls throughout:
  x loaded fp32 (residual) -> gpsimd cast bf16 -> TensorE transpose ->
  DVE copy to xT fp8.
  q/k/v projections fp8 DoubleRow (contraction 2x128 per pass); q/k bias
  via ScalarE Identity epilogue (per-partition), v bias via ones-row
  matmul, v copy via ScalarE.
  Scores computed TRANSPOSED (sT[s,t]) with u-pairs in one DR matmul;
  exp on ScalarE over 2-bank psum groups (scale 1/sqrt(U), shift -2 for
  fp8 range), p stored fp8.
  PV with v as stationary (s-pairs) emits aT[u,t] directly -- no
  attention-matrix transposes.  Row sums via all-ones DR matmul
  replicated across partitions; 1/D (fast reciprocal) folded into the
  aT psum->sbuf copy.  Output projection is a single DR matmul; residual
  x+ba is pre-combined on gpsimd so the epilogue is one DVE add.
"""

import numpy as np

import concourse.bass as bass
import concourse.mybir as mybir
import concourse.tile as tile
from concourse import bacc
from concourse.bass import ds, ts
from concourse.bass_utils import run_bass_kernel_spmd
from concourse.masks import make_identity

F32 = mybir.dt.float32
BF16 = mybir.dt.bfloat16
F8 = mybir.dt.float8e4
AF = mybir.ActivationFunctionType
DR = mybir.MatmulPerfMode.DoubleRow

B, T, C, U, P = 8, 2048, 512, 256, 128
TC = T // P   # 16 row tiles
CCH = C // P  # 4 c-chunks
UCH = 2       # u-chunks
TBLK = 512    # t-block for attention
NTB = T // TBLK
SCALE = 1.0 / float(np.sqrt(U))
SHIFT = -2.0  # exp(x*SCALE + SHIFT): keeps p in fp8e4 range

USE_DR = True

_cache = {}


def _dr_matmul(nc, out, lhsT3, rhs3, start, stop):
    """One fp8 DoubleRow matmul over [K,2,M]x[K,2,N], or two plain matmuls."""
    if USE_DR:
        nc.tensor.matmul(out, lhsT=lhsT3, rhs=rhs3, start=start, stop=stop,
                         perf_mode=DR)
    else:
        nc.tensor.matmul(out, lhsT=lhsT3[:, 0], rhs=rhs3[:, 0],
                         start=start, stop=False)
        nc.tensor.matmul(out, lhsT=lhsT3[:, 1], rhs=rhs3[:, 1],
                         start=False, stop=stop)


def _build_kernel(tc):
    nc = tc.nc
    x = nc.dram_tensor("x", [T, C], F32, kind="ExternalInput").ap()
    Wq = nc.dram_tensor("Wq", [C, U], F32, kind="ExternalInput").ap()
    bq = nc.dram_tensor("bq", [U], F32, kind="ExternalInput").ap()
    Wk = nc.dram_tensor("Wk", [C, U], F32, kind="ExternalInput").ap()
    bk = nc.dram_tensor("bk", [U], F32, kind="ExternalInput").ap()
    Wv = nc.dram_tensor("Wv", [C, U], F32, kind="ExternalInput").ap()
    bv = nc.dram_tensor("bv", [U], F32, kind="ExternalInput").ap()
    Wa = nc.dram_tensor("Wa", [U, C], F32, kind="ExternalInput").ap()
    ba = nc.dram_tensor("ba", [C], F32, kind="ExternalInput").ap()
    out = nc.dram_tensor("out", [T, C], F32, kind="ExternalOutput").ap()

    consts = tc.alloc_tile_pool(name="consts", bufs=1)
    persist = tc.alloc_tile_pool(name="persist", bufs=1)

    # --- constants ---
    identity = consts.tile([P, P], BF16)
    make_identity(nc, identity)
    ones_row = consts.tile([1, P], F8)
    nc.vector.memset(ones_row, 1.0)
    ones_pair = consts.tile([P, 2, P], F8)
    nc.vector.memset(ones_pair, 1.0)
    shift_col = consts.tile([P, 1], F32)
    nc.vector.memset(shift_col, SHIFT)

    # --- persistent tensors ---
    x32_sb = persist.tile([P, TC, C], F32)    # residual (fp32)
    x_sb = persist.tile([P, TC, C], BF16)     # transpose source
    xT_sb = persist.tile([P, CCH, T], F8)     # x^T  (c on partitions)
    qT_sb = persist.tile([P, UCH, T], F8)     # q^T  (u on partitions)
    kT_sb = persist.tile([P, UCH, T], F8)
    v_sb = persist.tile([P, TC, U], F8)       # v row-major (s, u)
    aT_sb = persist.tile([P, UCH, T], F8)     # a^T normalized
    p_sb = [persist.tile([P, TC, TBLK], F8, name=f"p{i}") for i in range(NTB)]

    # x loads (fp32, HW DGE, alternate queues); bf16 casts spread over
    # DVE / ScalarE / GpSimd
    for tt in range(TC):
        eng = nc.sync if tt % 2 == 0 else nc.scalar
        eng.dma_start(out=x32_sb[:, tt, :], in_=x[ts(tt, P), :])
    for tt in range(TC):
        if tt % 4 == 3:
            nc.scalar.copy(out=x_sb[:, tt, :], in_=x32_sb[:, tt, :])
        elif tt % 4 == 1:
            nc.gpsimd.tensor_copy(out=x_sb[:, tt, :], in_=x32_sb[:, tt, :])
        else:
            nc.vector.tensor_copy(out=x_sb[:, tt, :], in_=x32_sb[:, tt, :])

    # --- weights: fp32 DMA (HW DGE, vector queue) + DVE cast -> fp8 ---
    # layouts: W* [c_lo, cc, u] so cc-pairs (0,1),(2,3) give c/c+128 pairs;
    # Wa [u_lo, uc, c] matching aT's (u_lo, uc) partition layout.
    Wq_s = consts.tile([P, CCH, U], F8)
    Wk_s = consts.tile([P, CCH, U], F8)
    Wv_s = consts.tile([P, CCH, U], F8)
    Wa_s = consts.tile([P, UCH, C], F8)
    bq_sb = consts.tile([P, UCH], F32)
    nc.sync.dma_start(out=bq_sb, in_=bq.rearrange("(uc p) -> p uc", p=P))
    bk_sb = consts.tile([P, UCH], F32)
    nc.sync.dma_start(out=bk_sb, in_=bk.rearrange("(uc p) -> p uc", p=P))
    bv_row = consts.tile([1, U], F8)
    ba_row = consts.tile([1, C], F8)

    with tc.tile_pool(name="wstage", bufs=1) as wstage:
        Wv_f = wstage.tile([P, CCH, U], F32, tag="wv")
        nc.sync.dma_start(out=Wv_f, in_=Wv.rearrange("(cc p) u -> p cc u", p=P))
        nc.vector.tensor_copy(out=Wv_s, in_=Wv_f)
        Wk_f = wstage.tile([P, CCH, U], F32, tag="wk")
        nc.sync.dma_start(out=Wk_f, in_=Wk.rearrange("(cc p) u -> p cc u", p=P))
        nc.vector.tensor_copy(out=Wk_s, in_=Wk_f)
        Wq_f = wstage.tile([P, CCH, U], F32, tag="wq")
        nc.sync.dma_start(out=Wq_f, in_=Wq.rearrange("(cc p) u -> p cc u", p=P))
        nc.vector.tensor_copy(out=Wq_s, in_=Wq_f)
        Wa_f = wstage.tile([P, UCH, C], F32, tag="wa")
        nc.sync.dma_start(out=Wa_f, in_=Wa.rearrange("(uc p) c -> p uc c", p=P))
        nc.vector.tensor_copy(out=Wa_s, in_=Wa_f)
        bv_f = wstage.tile([1, U], F32, tag="bv")
        nc.sync.dma_start(out=bv_f, in_=bv[None, :])
        nc.vector.tensor_copy(out=bv_row, in_=bv_f)
        ba_f = wstage.tile([1, C], F32, tag="ba")
        nc.sync.dma_start(out=ba_f, in_=ba[None, :])
        nc.vector.tensor_copy(out=ba_row, in_=ba_f)

        # HAM warmup while DMAs land
        with tc.tile_pool(name="warm", bufs=1, space="PSUM") as warm_pool:
            wtile = warm_pool.tile([P, P], F32, name="warmup")
            for i in range(36):
                nc.tensor.matmul(wtile, lhsT=identity, rhs=identity,
                                 start=(i == 0), stop=(i == 35))

        # --- phase 1+2: transpose + projections, per t-block group ---
        with tc.tile_pool(name="xbf", bufs=4) as xbf_pool, \
             tc.tile_pool(name="tpsum", bufs=2, space="PSUM") as tpsum, \
             tc.tile_pool(name="wpsum", bufs=2, space="PSUM") as wpsum, \
             tc.tile_pool(name="vpsum", bufs=2, space="PSUM") as vpsum:
            for g in range(NTB):
                for half in range(2):
                    tt0 = 4 * g + 2 * half
                    tps = tpsum.tile([P, CCH, 2 * P], BF16, tag="tps")
                    for i in range(2):
                        for cc in range(CCH):
                            nc.tensor.transpose(
                                tps[:, cc, ts(i, P)],
                                x_sb[:, tt0 + i, ts(cc, P)], identity)
                    nc.vector.tensor_copy(out=xT_sb[:, :, ds(tt0 * P, 2 * P)],
                                          in_=tps)
                    for tt in (tt0, tt0 + 1):
                        vps = vpsum.tile([P, U], F32, tag="vps")
                        for cp in range(2):
                            _dr_matmul(nc, vps,
                                       xT_sb[:, ds(2 * cp, 2), ts(tt, P)],
                                       Wv_s[:, ds(2 * cp, 2), :],
                                       start=(cp == 0), stop=False)
                        nc.tensor.matmul(vps, lhsT=ones_row, rhs=bv_row,
                                         start=False, stop=True)
                        nc.scalar.copy(out=v_sb[:, tt, :], in_=vps)
                # q/k projections for this 512-wide t block
                for (W_s, b_sb, dst) in ((Wk_s, bk_sb, kT_sb),
                                         (Wq_s, bq_sb, qT_sb)):
                    for uc in range(UCH):
                        wps = wpsum.tile([P, TBLK], F32, tag="wps")
                        for cp in range(2):
                            _dr_matmul(nc, wps,
                                       W_s[:, ds(2 * cp, 2), ts(uc, P)],
                                       xT_sb[:, ds(2 * cp, 2), ts(g, TBLK)],
                                       start=(cp == 0), stop=(cp == 1))
                        nc.scalar.activation(out=dst[:, uc, ts(g, TBLK)],
                                             in_=wps, func=AF.Identity,
                                             bias=b_sb[:, uc:uc + 1], scale=1.0)


    # --- phase 3: attention ---
    sps_pool = tc.alloc_tile_pool(name="sps", bufs=2, space="PSUM")
    pv_pool = tc.alloc_tile_pool(name="pvps", bufs=2, space="PSUM")
    d_pool = tc.alloc_tile_pool(name="dps", bufs=1, space="PSUM")
    y_psum = tc.alloc_tile_pool(name="ypsum", bufs=1, space="PSUM")
    rcp_pool = tc.alloc_tile_pool(name="rcp", bufs=2)
    y_pool = tc.alloc_tile_pool(name="y", bufs=3)

    def outproj(tb, tsl, alt_pool=None):
        row0 = tb * TBLK + tsl * P
        if alt_pool is not None:
            yps = alt_pool.tile([P, C], F32, tag="d", name=f"yalt{tsl}")
        else:
            yps = y_psum.tile([P, C], F32, tag="yps")
        _dr_matmul(nc, yps, aT_sb[:, :, ds(row0, P)], Wa_s,
                   start=True, stop=False)
        nc.tensor.matmul(yps, lhsT=ones_row, rhs=ba_row,
                         start=False, stop=True)
        y_sb = y_pool.tile([P, C], F32, tag="ysb")
        nc.vector.tensor_add(out=y_sb, in0=yps,
                             in1=x32_sb[:, tb * NTB + tsl, :])
        nc.sync.dma_start(out=out[ds(row0, P), :], in_=y_sb)

    def pv_pair(tb, j, apsT, drep):
        rhs_p = p_sb[tb][:, ds(2 * j, 2), :]
        for uc in range(UCH):
            _dr_matmul(nc, apsT[uc],
                       v_sb[:, ds(2 * j, 2), ts(uc, P)], rhs_p,
                       start=(j == 0), stop=(j == 7))
        _dr_matmul(nc, drep, ones_pair, rhs_p,
                   start=(j == 0), stop=(j == 7))

    for tb in range(NTB):
        apsT = [pv_pool.tile([P, TBLK], F32, tag="pv",
                             name=f"apsT{tb}_{uc}") for uc in range(UCH)]
        drep = d_pool.tile([P, TBLK], F32, tag="d", name=f"drep{tb}")
        for j in range(8):
            sps_t = sps_pool.tile([P, 2, TBLK], F32, tag="sps")
            for h in range(2):
                _dr_matmul(nc, sps_t[:, h, :],
                           kT_sb[:, :, ts(2 * j + h, P)],
                           qT_sb[:, :, ts(tb, TBLK)],
                           start=True, stop=True)
            nc.scalar.activation(out=p_sb[tb][:, ds(2 * j, 2), :],
                                 in_=sps_t, func=AF.Exp,
                                 bias=shift_col, scale=SCALE)
            if j >= 2:
                pv_pair(tb, j - 2, apsT, drep)
            if tb > 0 and 2 <= j < 6:
                outproj(tb - 1, j - 2)
        pv_pair(tb, 6, apsT, drep)
        pv_pair(tb, 7, apsT, drep)
        # drain: fast reciprocal of replicated row sums, normalize into aT
        rcp = rcp_pool.tile([P, TBLK], F32, tag="rcp")
        nc.vector.reciprocal_approx_fast(out=rcp, in_=drep)
        for uc in range(UCH):
            nc.vector.tensor_mul(out=aT_sb[:, uc, ts(tb, TBLK)],
                                 in0=apsT[uc], in1=rcp)
    for tsl in range(NTB):
        # alternate with the freed D bank so the tail pipelines
        outproj(NTB - 1, tsl, alt_pool=d_pool if tsl % 2 == 1 else None)

    for pool in (y_pool, rcp_pool, y_psum, d_pool, pv_pool, sps_pool,
                 persist, consts):
        pool.release()


def _get_nc():
    if "nc" not in _cache:
        nc = bacc.Bacc("TRN2", target_bir_lowering=False, debug=False)
        with tile.TileContext(nc) as tc:
            _build_kernel(tc)
        nc.compile()
        _cache["nc"] = nc
    return _cache["nc"]


def kernel(**inputs):
    nc = _get_nc()
    shared = {k: np.ascontiguousarray(np.asarray(v, dtype=np.float32))
              for k, v in inputs.items() if k != "x"}
    xs = np.ascontiguousarray(np.asarray(inputs["x"], dtype=np.float32))
    in_maps = [dict(shared, x=xs[b]) for b in range(B)]
    res = run_bass_kernel_spmd(nc, in_maps, core_ids=list(range(B)))
    return np.stack([res.results[b]["out"] for b in range(B)], axis=0)
